# revision 1
# baseline (speedup 1.0000x reference)
"""Trainium2 Bass kernel for nn_Net_49950469652573 (spiking ConvLSTM -> fc -> CfC).

Sharding: data-parallel over batch B=8 across 8 cores (1 sample/core) for the
ConvLSTM + fc1 + Leaky phase; spk2 is AllGathered and the tiny CfC scan (which
threads hidden state across batch elements) + the mem3 Leaky are computed
redundantly on every core, preserving the reference's sequential-over-batch
semantics.

Conv: fp8(e4m3) DoubleRow matmuls. The 17ch x 9ky x 9kx = 1377 taps are laid
out as 85 partition-PAIRS (p = ci*5 + k holds ky = 2k, 2k+1; ky=9 is a
zero-weight pad) with the kx shift expressed as a free-dim window offset, so
one [85, 2, 8, 64] DoubleRow matmul covers 170 taps at 0.5 cycles/row:
9 kx x 8 j-tiles = 72 matmuls per timestep (vs 2304 bf16 matmuls in v1).

Data flow per timestep (all-SBUF recurrent path, no DRAM image):
  img[17, 5760] fp8 (ci planes) --1 DMA--> r85[85, 2, 64, 72] fp8 (pair rows)
  -> matmuls -> psum[64=(gate,co), 4, 512] f32 x2 (q = j-half)
  -> DMA psum->DRAM->SBUF gallf[128=(j,co), (gate,x)] f32 (partition expand)
  -> sigmoid[128,1536]/tanh[128,512] -> DVE LSTM math (bf16) -> mem1 fp8
  -> 8 plane DMAs back into img.
"""

import numpy as np
import ml_dtypes

import concourse.bacc as bacc
import concourse.bass as bass
import concourse.tile as tile
from concourse import mybir
from concourse.bass_utils import run_bass_kernel_spmd

F32 = mybir.dt.float32
BF16 = mybir.dt.bfloat16
FP8 = mybir.dt.float8e4
NPBF16 = ml_dtypes.bfloat16
NPFP8 = ml_dtypes.float8_e4m3
AOP = mybir.AluOpType
AF = mybir.ActivationFunctionType
DR = mybir.MatmulPerfMode.DoubleRow

N_CORES = 8
T = 16
CO = 16            # conv out channels per gate
WP = 72            # padded row width
IMROWS = 80        # padded image rows
IMLEN = IMROWS * WP  # 5760
NR = 64 * WP       # 4608 (one R row half)
NSENS, NINTER, NCMD, NMOTOR = 75, 9, 5, 6
BETA = 0.9

# CfC layer dims (see v1)
CFC = [
    dict(d=84, h=9),
    dict(d=14, h=5),
    dict(d=11, h=6),
]


def build(debug=False, reps=1, sim=False, phases=3, ablate=(), perq=False):
    nc = bacc.Bacc("TRN2", target_bir_lowering=False, debug=False,
                   num_devices=1 if sim else N_CORES)

    # ---------------- external inputs ----------------
    xin = nc.dram_tensor("xin", [T, 4096], F32, kind="ExternalInput")
    wc_d = nc.dram_tensor("wc", [85, 9 * 128], FP8, kind="ExternalInput")
    fw_d = nc.dram_tensor("fw", [128, 128 * 75], BF16, kind="ExternalInput")
    fc1b_d = nc.dram_tensor("fc1b", [75, 1], F32, kind="ExternalInput")
    ident_d = nc.dram_tensor("ident", [128, 128], BF16, kind="ExternalInput")
    # CfC: host-fused weights [d, 96] = [f1*mk @0 | f2*mk @32 | ta+tb @64]
    # (32-aligned M blocks so engine ops on psum slices are base-aligned),
    # tanh bias [64,1] (b1@0, b2@32), sigmoid bias [32,1]
    cfc_d = {}
    for l, c in enumerate(CFC):
        d = c["d"]
        cfc_d[(l, "fus")] = nc.dram_tensor(f"cfus{l}", [d, 96], BF16,
                                           kind="ExternalInput")
        for nm in ("b1", "b2", "bs"):
            cfc_d[(l, nm)] = nc.dram_tensor(f"c{nm}{l}", [16, 1], F32,
                                            kind="ExternalInput")

    ospk = nc.dram_tensor("ospk", [T * 8 * 6], F32, kind="ExternalOutput")
    omem = nc.dram_tensor("omem", [T * 8 * 6], F32, kind="ExternalOutput")
    dbg = {}
    if debug:
        dbg["d_mem1"] = nc.dram_tensor("d_mem1", [128, 512], F32, kind="ExternalOutput")
        dbg["d_syn1"] = nc.dram_tensor("d_syn1", [128, 512], F32, kind="ExternalOutput")
        dbg["d_spk1"] = nc.dram_tensor("d_spk1", [128, T * 128], F32, kind="ExternalOutput")
        dbg["d_cur2"] = nc.dram_tensor("d_cur2", [75, 16], F32, kind="ExternalOutput")
        dbg["d_mem2"] = nc.dram_tensor("d_mem2", [75, 16], F32, kind="ExternalOutput")
        dbg["d_r85"] = nc.dram_tensor("d_r85", [85, 2 * NR], FP8, kind="ExternalOutput")
        dbg["d_gallf"] = nc.dram_tensor("d_gallf", [128, 2048], BF16, kind="ExternalOutput")
        dbg["d_ev0"] = nc.dram_tensor("d_ev0", [64, 4096], BF16, kind="ExternalOutput")
        dbg["d_img"] = nc.dram_tensor("d_img", [16, IMLEN], FP8, kind="ExternalOutput")
        dbg["d_m1p"] = nc.dram_tensor("d_m1p", [128, 8 * WP], FP8, kind="ExternalOutput")
        dbg["d_r85b"] = nc.dram_tensor("d_r85b", [85, 2 * NR], FP8, kind="ExternalOutput")
        dbg["d_pool"] = nc.dram_tensor("d_pool", [128, 128], FP8, kind="ExternalOutput")
        for tt in (1, 2, 4, 8, 12):
            dbg[f"d_m1p{tt}"] = nc.dram_tensor(f"d_m1p{tt}", [128, 8 * WP], FP8,
                                               kind="ExternalOutput")

    gin = nc.dram_tensor("gin", [75 * 16], F32)
    gout = nc.dram_tensor("gout", [8 * 75 * 16], F32, addr_space="Shared")

    with tile.TileContext(nc) as tc:
        with (
            tc.tile_pool(name="persist", bufs=1) as pp,
            tc.tile_pool(name="work", bufs=3) as wk,
            tc.tile_pool(name="psum", bufs=2, space="PSUM") as psp,
        ):
            # ---------------- persistent SBUF ----------------
            wconv = pp.tile([85, 9 * 128], FP8)
            fw = pp.tile([128, 128 * 75], BF16)
            fc1b = pp.tile([75, 1], F32)
            ident = pp.tile([128, 128], BF16)
            xpad8 = pp.tile([T, IMLEN], FP8)
            img = pp.tile([16, IMLEN], FP8)
            r85s = [pp.tile([85, 2 * NR], FP8, name=f"r85_{i}") for i in range(2)]
            syn1 = pp.tile([128, 512], BF16)
            mem1ps = [pp.tile([128, 8 * WP], FP8, name=f"mem1p_{i}")
                      for i in range(2)]          # mem1, rows of 72 w/ pads
            spk1_all = pp.tile([128, T * 128], BF16)
            spk1t = pp.tile([128, T * 128], BF16)
            cur2 = pp.tile([75, 16], F32)
            mem2 = pp.tile([75, 1], F32)
            spk2 = pp.tile([75, 16], F32)
            # CfC persistent state
            cw = {}
            for l, c in enumerate(CFC):
                d, h = c["d"], c["h"]
                cw[(l, "fus")] = pp.tile([d, 96], BF16, name=f"cw_fus{l}")
                for nm in ("b1", "b2", "bs"):
                    cw[(l, nm)] = pp.tile([16, 1], F32, name=f"cw_{nm}{l}")
            rhs0 = pp.tile([84, 8 * 16], BF16)
            rhs1 = pp.tile([14, 8 * 16], BF16)
            rhs2 = pp.tile([11, 8 * 16], BF16)
            cur3 = pp.tile([6, 8 * 16], F32)
            mem3 = pp.tile([6, 8], F32)
            om = pp.tile([6, T * 8], F32)
            osb = pp.tile([6, T * 8], F32)

            # ---------------- load constants ----------------
            nc.sync.dma_start(out=wconv[:], in_=wc_d[:])
            nc.sync.dma_start(out=fw[:], in_=fw_d[:])
            nc.sync.dma_start(out=fc1b[:], in_=fc1b_d[:])
            nc.sync.dma_start(out=ident[:], in_=ident_d[:])
            for l, c in enumerate(CFC):
                for nm in ("fus", "b1", "b2", "bs"):
                    nc.sync.dma_start(out=cw[(l, nm)][:], in_=cfc_d[(l, nm)][:])

            # x -> padded fp8 images
            xsb = pp.tile([T, 4096], F32)
            nc.sync.dma_start(out=xsb[:], in_=xin[:])
            nc.vector.memset(xpad8[:], 0.0)
            xpv = xpad8[:].rearrange("t (y w) -> t y w", w=WP)
            nc.vector.tensor_copy(
                out=xpv[:, 8:72, 4:68],
                in_=xsb[:].rearrange("t (y x) -> t y x", x=64),
            )


            # weight view per kx: [85, 2, 64]
            wcv = wconv[:].rearrange("p (kx i m) -> p kx i m", kx=9, i=2)
            img_ap = img[:]
            xp_ap = xpad8[:]

            for rep in range(reps):
                # ---------------- state init ----------------
                nc.vector.memset(img[:], 0.0)
                nc.vector.memset(mem1ps[0][:], 0.0)
                nc.vector.memset(mem1ps[1][:], 0.0)
                nc.vector.memset(syn1[:], 0.0)
                nc.vector.memset(mem2[:], 0.0)
                nc.vector.memset(mem3[:], 0.0)

                # ---------------- phase 1: ConvLSTM over T ----------------
                for t in range(T):
                    r85 = r85s[t % 2]
                    mem1p = mem1ps[t % 2]
                    r_ap = r85[:]
                    r85v = r85[:].rearrange("p (i y x) -> p i y x", i=2, x=WP)
                    if "r" not in ablate:
                        # R-main halves: cols [0,2304) serve j-tiles 0..3,
                        # [2304,4608) serve 4..7 -> mm j0 starts after half A
                        for h, eng in ((0, nc.sync), (1, nc.scalar)):
                            off = h * 2304
                            src = bass.AP(tensor=img_ap.tensor,
                                          offset=img_ap.offset + 4 * WP + off,
                                          ap=[[IMLEN, 16], [WP, 10], [1, 2304]])
                            dst = bass.AP(tensor=r_ap.tensor,
                                          offset=r_ap.offset + off,
                                          ap=[[2 * NR, 80], [NR, 2], [1, 2304]])
                            eng.dma_start(out=dst, in_=src)
                        # R-x: xpad8[t] -> r85 rows 80..84 (own queue, early)
                        src = bass.AP(tensor=xp_ap.tensor,
                                      offset=xp_ap.offset + t * IMLEN + 4 * WP,
                                      ap=[[IMLEN, 1], [WP, 10], [1, NR]])
                        dst = bass.AP(tensor=r_ap.tensor,
                                      offset=r_ap.offset + 80 * 2 * NR,
                                      ap=[[2 * NR, 5], [NR, 2], [1, NR]])
                        nc.gpsimd.dma_start(out=dst, in_=src)

                    # one pass over all 8 j-tiles, two 4-bank PSUM tiles;
                    # evacuate in 4 pieces interleaved with the matmul burst
                    ev = wk.tile([64, 8 * 512], BF16, tag="ev")
                    for half in range(2):
                        ps = psp.tile([64, 4 * 512], F32, tag="ps")
                        psv = ps[:].rearrange("p (j x) -> p j x", j=4)
                        for jj in range(4):
                            j = half * 4 + jj
                            for kx in range(9):
                                rhs = bass.AP(
                                    tensor=r85v.tensor,
                                    offset=r85v.offset + j * 8 * WP + kx,
                                    ap=[[2 * NR, 85], [NR, 2], [WP, 8], [1, 64]])
                                nc.tensor.matmul(
                                    psv[:, jj, :], wcv[:, kx, :, :], rhs,
                                    start=(kx == 0), stop=(kx == 8),
                                    perf_mode=DR)
                            if jj % 2 == 1:
                                sl = slice((j - 1) * 512, (j + 1) * 512)
                                pl = slice((jj - 1) * 512, (jj + 1) * 512)
                                if half == 0:
                                    nc.vector.tensor_copy(out=ev[:, sl],
                                                          in_=ps[:, pl])
                                else:
                                    nc.scalar.activation(out=ev[:, sl],
                                                         in_=ps[:, pl],
                                                         func=AF.Copy)
                    # hop: one SBUF->SBUF DMA per gate. gallf partition
                    # p = co*8 + j, so dst [:, g-col] and src ev[16g:16g+16]
                    # both iterate (co, j, x). Order: gi, gf first (feed the
                    # first sigmoid), gg next (tanh), go last.
                    gallf = wk.tile([128, 4 * 512], BF16, tag="gal")
                    for n, g in enumerate((0, 1, 3, 2)):
                        eng = nc.sync if n % 2 == 0 else nc.scalar
                        eng.dma_start(
                            out=gallf[:, g * 512:(g + 1) * 512],
                            in_=ev[16 * g:16 * g + 16, :])

                    if "evac" in ablate:
                        continue
                    # activations: free layout (g, x); g: 0=i 1=f 2=o 3=g
                    gs = wk.tile([128, 3 * 512], BF16, tag="gs")
                    gt = wk.tile([128, 512], BF16, tag="gt")
                    tsn = wk.tile([128, 512], BF16, tag="tsn")
                    tmp = wk.tile([128, 512], BF16, tag="tmp")
                    nc.scalar.activation(out=gs[:, 0:1024], in_=gallf[:, 0:1024],
                                         func=AF.Sigmoid)   # sig(gi), sig(gf)
                    nc.scalar.activation(out=gt[:], in_=gallf[:, 1536:2048],
                                         func=AF.Tanh)      # tanh(gg)
                    nc.vector.tensor_mul(syn1[:], syn1[:], gs[:, 512:1024])
                    nc.vector.tensor_mul(tmp[:], gs[:, 0:512], gt[:])
                    nc.scalar.activation(out=gs[:, 1024:1536],
                                         in_=gallf[:, 1024:1536],
                                         func=AF.Sigmoid)   # sig(go)
                    nc.vector.tensor_add(syn1[:], syn1[:], tmp[:])
                    m1w = mem1p[:].rearrange("p (a w) -> p a w", w=WP)
                    nc.scalar.activation(out=tsn[:], in_=syn1[:], func=AF.Tanh)
                    nc.vector.tensor_mul(
                        m1w[:, :, 4:68],
                        gs[:, 1024:1536].rearrange("p (a x) -> p a x", a=8),
                        tsn[:].rearrange("p (a x) -> p a x", a=8))
                    # mem1p -> img: one DMA; src [128=(co,j), 576] iterates
                    # (co, j, row); dst plane co rows 8..71 contiguous.
                    if "m1pad" not in ablate:
                        dst = bass.AP(
                            tensor=img_ap.tensor,
                            offset=img_ap.offset + 8 * WP,
                            ap=[[IMLEN, 16], [1, NR]])
                        nc.sync.dma_start(out=dst, in_=mem1p[:])

                    # pooling + spike (fp8 in, bf16 spike out)
                    m1i = m1w[:, :, 4:68].rearrange("p a (x two) -> p a x two",
                                                    two=2)
                    px = wk.tile([128, 8 * 32], FP8, tag="px")
                    pxv = px[:].rearrange("p (a x) -> p a x", a=8)
                    nc.vector.tensor_max(pxv, m1i[:, :, :, 0], m1i[:, :, :, 1])
                    px2 = pxv.rearrange("p (b two) x -> p b two x", two=2)
                    pool = wk.tile([128, 4 * 32], FP8, tag="pool")
                    plv = pool[:].rearrange("p (b x) -> p b x", b=4)
                    nc.vector.tensor_max(plv, px2[:, :, 0, :], px2[:, :, 1, :])
                    nc.vector.tensor_scalar(
                        out=spk1_all[:, t * 128:(t + 1) * 128],
                        in0=pool[:], scalar1=1.0, scalar2=None, op0=AOP.is_gt)

                    if debug and rep == reps - 1 and t == 0:
                        nc.sync.dma_start(out=dbg["d_r85"][:], in_=r85[:])
                        nc.sync.dma_start(out=dbg["d_gallf"][:], in_=gallf[:])
                        nc.sync.dma_start(out=dbg["d_ev0"][:], in_=ev[:])
                        nc.sync.dma_start(out=dbg["d_img"][:], in_=img[:])
                        nc.sync.dma_start(out=dbg["d_m1p"][:], in_=mem1p[:])
                        nc.sync.dma_start(out=dbg["d_pool"][:], in_=pool[:])
                    if debug and rep == reps - 1 and t == 1:
                        nc.sync.dma_start(out=dbg["d_r85b"][:], in_=r85[:])
                    if debug and rep == reps - 1 and t in (1, 2, 4, 8, 12):
                        nc.sync.dma_start(out=dbg[f"d_m1p{t}"][:], in_=mem1p[:])

                # ---------------- phase 1.5: fc1 + mem2 ----------------
                if phases < 2:
                    continue
                for t in range(T):
                    pt = psp.tile([128, 128], BF16, tag="ps")
                    nc.tensor.transpose(pt[:], spk1_all[:, t * 128:(t + 1) * 128],
                                        ident[:])
                    nc.vector.tensor_copy(out=spk1t[:, t * 128:(t + 1) * 128],
                                          in_=pt[:])
                s1tv = spk1t[:].rearrange("p (t k) -> p t k", k=128)
                c2ps = psp.tile([75, 16], F32, tag="ps")
                for b in range(128):
                    nc.tensor.matmul(c2ps[:], fw[:, b * 75:(b + 1) * 75],
                                     s1tv[:, :, b],
                                     start=(b == 0), stop=(b == 127))
                nc.vector.tensor_scalar(out=cur2[:], in0=c2ps[:],
                                        scalar1=fc1b[:], scalar2=None, op0=AOP.add)
                mem2h = pp.tile([75, 16], F32, name="mem2h") if debug else None
                r2t = wk.tile([75, 1], F32, tag="r2t")
                for t in range(T):
                    nc.vector.tensor_scalar(out=r2t[:], in0=mem2[:],
                                            scalar1=1.0, scalar2=None, op0=AOP.is_gt)
                    nc.vector.tensor_sub(r2t[:], cur2[:, t:t + 1], r2t[:])
                    nc.vector.tensor_scalar_mul(mem2[:], mem2[:], BETA)
                    nc.vector.tensor_add(mem2[:], mem2[:], r2t[:])
                    nc.vector.tensor_scalar(out=spk2[:, t:t + 1], in0=mem2[:],
                                            scalar1=1.0, scalar2=None, op0=AOP.is_gt)
                    if debug and mem2h is not None:
                        nc.vector.tensor_copy(out=mem2h[:, t:t + 1], in_=mem2[:])

                if debug and rep == reps - 1:
                    d_m1 = pp.tile([128, 512], F32, name="d_m1")
                    d_s1 = pp.tile([128, 512], F32, name="d_s1")
                    d_sp = pp.tile([128, T * 128], F32, name="d_sp")
                    m1wv = mem1p[:].rearrange("p (a w) -> p a w", w=WP)
                    nc.vector.tensor_copy(
                        out=d_m1[:].rearrange("p (a x) -> p a x", a=8),
                        in_=m1wv[:, :, 4:68])
                    nc.vector.tensor_copy(out=d_s1[:], in_=syn1[:])
                    nc.vector.tensor_copy(out=d_sp[:], in_=spk1_all[:])
                    nc.sync.dma_start(out=dbg["d_mem1"][:], in_=d_m1[:])
                    nc.sync.dma_start(out=dbg["d_syn1"][:], in_=d_s1[:])
                    nc.sync.dma_start(out=dbg["d_spk1"][:], in_=d_sp[:])
                    nc.sync.dma_start(out=dbg["d_cur2"][:], in_=cur2[:])
                    nc.sync.dma_start(out=dbg["d_mem2"][:], in_=mem2h[:])

                # ---------------- gather spk2 ----------------
                if phases < 3:
                    continue
                nc.sync.dma_start(
                    out=bass.AP(tensor=gin, offset=0, ap=[[16, 75], [1, 16]]),
                    in_=spk2[:])
                if sim:
                    for bb in range(8):
                        nc.sync.dma_start(out=gout[bb * 1200:(bb + 1) * 1200],
                                          in_=gin[:])
                else:
                    nc.gpsimd.collective_compute(
                        "AllGather", AOP.bypass,
                        replica_groups=[list(range(N_CORES))],
                        ins=[gin[:]], outs=[gout[:]])
                nc.gpsimd.dma_start(
                    out=rhs0[9:84, :].rearrange("p (b t) -> p b t", t=16),
                    in_=bass.AP(tensor=gout, offset=0,
                                ap=[[16, 75], [1200, 8], [1, 16]]))

                # ---------------- phase 2: CfC + mem3 ----------------
                nc.vector.memset(rhs0[0:9, 0:16], 0.0)
                nc.vector.memset(rhs1[0:14, 0:16], 0.0)
                nc.vector.memset(rhs2[0:11, 0:16], 0.0)
                rhs = [rhs0, rhs1, rhs2]
                for b in range(8):
                    col = slice(b * 16, (b + 1) * 16)
                    ncol = slice((b + 1) * 16, (b + 2) * 16)
                    for l, c in enumerate(CFC):
                        d, h = c["d"], c["h"]
                        pc = psp.tile([96, 16], F32, tag="ps")
                        rr = rhs[l][0:d, col]
                        nc.tensor.matmul(pc[:, :], cw[(l, "fus")][:], rr,
                                         start=True, stop=True)
                        f1t = wk.tile([16, 16], F32, tag=f"f1_{l}")
                        f2t = wk.tile([16, 16], F32, tag=f"f2_{l}")
                        ti = wk.tile([16, 16], F32, tag=f"ti_{l}")
                        nc.scalar.activation(out=f1t[0:h, :], in_=pc[0:h, :],
                                             func=AF.Tanh,
                                             bias=cw[(l, "b1")][0:h, :])
                        nc.scalar.activation(out=f2t[0:h, :],
                                             in_=pc[32:32 + h, :],
                                             func=AF.Tanh,
                                             bias=cw[(l, "b2")][0:h, :])
                        nc.scalar.activation(out=ti[0:h, :],
                                             in_=pc[64:64 + h, :],
                                             func=AF.Sigmoid,
                                             bias=cw[(l, "bs")][0:h, :])
                        # h' = f1 + ti*(f2 - f1)
                        f1 = f1t[0:h, :]
                        f2 = f2t[0:h, :]
                        nc.vector.tensor_sub(f2, f2, f1)
                        nc.vector.tensor_mul(f2, f2, ti[0:h, :])
                        if l == 0:
                            nc.vector.tensor_add(rhs1[0:9, col], f1, f2)
                            if b < 7:
                                nc.vector.tensor_add(rhs0[0:9, ncol], f1, f2)
                        elif l == 1:
                            nc.vector.tensor_add(rhs2[0:5, col], f1, f2)
                            if b < 7:
                                nc.sync.dma_start(out=rhs1[9:14, ncol],
                                                  in_=rhs2[0:5, col])
                        else:
                            nc.vector.tensor_add(cur3[:, col], f1, f2)
                            if b < 7:
                                nc.gpsimd.dma_start(out=rhs2[5:11, ncol],
                                                    in_=cur3[0:6, col])

                # mem3 Leaky over t
                c3v = cur3[:].rearrange("p (b t) -> p b t", t=16)
                r3t = wk.tile([6, 8], F32, tag="r3t")
                for t in range(T):
                    nc.vector.tensor_scalar(out=r3t[:], in0=mem3[:],
                                            scalar1=1.0, scalar2=None, op0=AOP.is_gt)
                    nc.vector.tensor_sub(r3t[:], c3v[:, :, t], r3t[:])
                    nc.vector.tensor_scalar_mul(mem3[:], mem3[:], BETA)
                    nc.vector.tensor_add(mem3[:], mem3[:], r3t[:])
                    nc.vector.tensor_copy(out=om[:, t * 8:(t + 1) * 8], in_=mem3[:])
                    nc.vector.tensor_scalar(out=osb[:, t * 8:(t + 1) * 8],
                                            in0=mem3[:],
                                            scalar1=1.0, scalar2=None, op0=AOP.is_gt)

                # outputs: [j p, (t b) f] -> flat t*48 + b*6 + j
                odst = [[1, 6], [48, T], [6, 8]]
                nc.sync.dma_start(out=bass.AP(tensor=omem, offset=0, ap=odst),
                                  in_=om[:])
                nc.sync.dma_start(out=bass.AP(tensor=ospk, offset=0, ap=odst),
                                  in_=osb[:])

    if sim:
        return nc
    nc.compile()
    return nc


# ---------------- host side ----------------

def _prep_shared(conv_w, fc1_w, fc1_b, cws, cbs, masks):
    conv_w = np.asarray(conv_w, np.float32)  # [64, 17, 9, 9]
    # wconv[p, kx, i, m]: p 0..79 = mem1 plane (ci = p//5 -> conv_w[:, ci+1],
    # k = p%5, ky = 2k+i); p 80..84 = x plane (conv_w[:, 0], k = p-80)
    wc = np.zeros((85, 9, 2, 64), np.float32)
    for p in range(85):
        src_ci = (p // 5) + 1 if p < 80 else 0
        k = p % 5 if p < 80 else p - 80
        for i in range(2):
            ky = 2 * k + i
            if ky > 8:
                continue
            # conv_w[m, src_ci, ky, kx] -> wc[p, kx, i, m]
            wc[p, :, i, :] = conv_w[:, src_ci, ky, :].T
    wc8 = np.ascontiguousarray(wc.reshape(85, 9 * 128)).astype(NPFP8)

    # fw[f, b, m]: b = co*8 + j (spk1 partition), f = a'*32 + xp,
    # feature = co*1024 + (4j + a')*32 + xp
    A = np.asarray(fc1_w, np.float32).T.reshape(16, 8, 4, 32, 75)  # co,j,a',xp,m
    fwt = A.transpose(2, 3, 0, 1, 4).reshape(128, 128, 75)  # (a',xp),(co,j),m
    fw = np.ascontiguousarray(fwt).reshape(128, 9600).astype(NPBF16)

    out = {
        "wc": wc8, "fw": fw,
        "fc1b": np.asarray(fc1_b, np.float32).reshape(75, 1),
        "ident": np.eye(128, dtype=NPBF16),
    }
    perms = [
        np.concatenate([np.arange(75, 84), np.arange(0, 75)]),
        np.arange(14),
        np.arange(11),
    ]
    hs = [9, 5, 6]
    for l in range(3):
        w4 = np.asarray(cws[l], np.float32)   # [4, h, d]
        b4 = np.asarray(cbs[l], np.float32)   # [4, h]
        mk = np.asarray(masks[l], np.float32)  # [h, d]
        p = perms[l]
        h = hs[l]
        d = w4.shape[2]
        fus = np.zeros((d, 96), np.float32)
        fus[:, 0:h] = (w4[0] * mk)[:, p].T
        fus[:, 32:32 + h] = (w4[1] * mk)[:, p].T
        fus[:, 64:64 + h] = (w4[2] + w4[3])[:, p].T
        out[f"cfus{l}"] = np.ascontiguousarray(fus).astype(NPBF16)
        for nm, vec in (("b1", b4[0]), ("b2", b4[1]), ("bs", b4[2] + b4[3])):
            b = np.zeros((16, 1), np.float32)
            b[0:h, 0] = vec
            out[f"c{nm}{l}"] = b
    return out


_CACHE = {}


def _get_nc(debug=False, reps=1):
    key = (debug, reps)
    if key not in _CACHE:
        _CACHE[key] = build(debug=debug, reps=reps)
    return _CACHE[key]


def make_in_maps(inputs, debug=False):
    shared = _prep_shared(
        inputs["conv_w"], inputs["fc1_w"], inputs["fc1_b"],
        [inputs["cfc_w0"], inputs["cfc_w1"], inputs["cfc_w2"]],
        [inputs["cfc_b0"], inputs["cfc_b1"], inputs["cfc_b2"]],
        [inputs["mask0"], inputs["mask1"], inputs["mask2"]],
    )
    x = np.asarray(inputs["x"], np.float32)  # [T, B, 1, 64, 64]
    in_maps = []
    for c in range(N_CORES):
        m = dict(shared)
        m["xin"] = np.ascontiguousarray(x[:, c, 0].reshape(T, 4096))
        in_maps.append(m)
    return in_maps


def kernel(**inputs):
    nc = _get_nc(debug=False, reps=1)
    in_maps = make_in_maps(inputs)
    res = run_bass_kernel_spmd(nc, in_maps, list(range(N_CORES)))
    r0 = res.results[0]
    spk3 = r0["ospk"].reshape(T, 8, 6).astype(np.float32)
    mem3 = r0["omem"].reshape(T, 8, 6).astype(np.float32)
    return spk3, mem3

